# revision 27
# baseline (speedup 1.0000x reference)
"""BLOOM attention block on 8 TRN2 NeuronCores.

Tensor-parallel over heads: core c computes heads 4c..4c+3 for both batches
(8 (b,h) pairs/core). Device math in bf16 with fp32 accumulation:

  phase 1: Q^T/K^T = (Wqkv_qk^T h)  [head-dim on partitions], V = h Wqkv_v
           hs loaded one group ahead via a single batched DMA on the gpsimd
           queue; wv streamed 2 tiles ahead on the scalar queue; weight
           stripes split across the sync/scalar descriptor queues.
  phase 2: per pair, causal-tiled scores in [k, q] layout. The per-q alibi
           term (-slope*q) is accumulated into the scores PSUM by a uniform
           rank-1 matmul (stationary = -slope/128 everywhere), the per-k term
           rides the Exp activation bias, so ScalarE exponentiates straight
           out of PSUM with no DVE pass. Causal mask applied post-exp as an
           affine_select fill-0 on GpSimd. Softmax denominator via two
           interleaved bf16 running sums on DVE + one per-chunk PE
           ones-reduce; 1/Z via reciprocal_approx_fast; normalization folded
           into the per-head ctx evac.
  phase 3: out_part = ctx Wd_c (per-core partial), PSUM evacuated on
           alternating Scalar/Vector engines into bf16, one store per
           128-token tile. PSUM pools are shared between phases (no pool
           transition barrier); all matmul work is sized to keep the chip
           out of its P0 power-throttle state where possible.

Host: shards/casts inputs, then out = residual + bd + sum_c out_part_c.
Self-contained: shapes hardcoded for B=2, S=2048, HID=4096, H=32, 8 cores.
"""

import math
from contextlib import ExitStack
from dataclasses import dataclass

import ml_dtypes
import numpy as np

import concourse.bacc as bacc
import concourse.mybir as mybir
import concourse.tile as tile
from concourse.bass import ts
from concourse.bass_utils import run_bass_kernel_spmd

F32 = mybir.dt.float32
BF16 = mybir.dt.bfloat16
AF = mybir.ActivationFunctionType
ALU = mybir.AluOpType
BF = ml_dtypes.bfloat16

N_CORES = 8


@dataclass(frozen=True)
class Cfg:
    B: int = 2
    S: int = 2048
    HID: int = 4096
    H_CORE: int = 4          # heads handled by this core
    HD: int = 128
    TG: int = 512            # phase-1 token group

    @property
    def TOKS(self):
        return self.B * self.S

    @property
    def NG(self):
        return self.TOKS // self.TG      # phase-1 groups

    @property
    def KT(self):
        return self.HID // 128          # hid tiles (contraction)

    @property
    def QK_CT(self):
        return 2 * self.H_CORE          # q+k coltiles

    @property
    def VC(self):
        return self.H_CORE * self.HD    # v columns (<= 512)

    @property
    def NQT(self):
        return self.S // 128            # q tiles per sequence

    @property
    def NPAIR(self):
        return self.B * self.H_CORE

    @property
    def MC(self):
        return self.VC // 128           # dense contraction chunks

    @property
    def CHQ(self):
        return self.S // 512            # q chunks (transposed attention)


FULL = Cfg()


def input_specs(cfg: Cfg):
    c = cfg
    return {
        "hsT": ([c.NG, 128, c.KT, c.TG], BF16),
        "wqkv_qk": ([c.QK_CT, 128, c.HID], BF16),
        "wqkv_v": ([128, c.KT * c.VC], BF16),
        "bias_qk": ([128, c.QK_CT], F32),
        "bqkv_v_rep": ([128, c.VC], BF16),
        "ramp_rep": ([128, 512], BF16),
        "nslope_mat": ([128, c.NPAIR * 128], BF16),
        "bias_kq": ([128, c.NPAIR * c.NQT * c.CHQ], F32),
        "wd": ([c.MC * 128, c.HID], BF16),
    }


def output_specs(cfg: Cfg):
    return {"out_part": ([cfg.TOKS, cfg.HID], BF16)}


def build(ctx: ExitStack, tc, outs, ins, cfg: Cfg):
    c = cfg
    nc = tc.nc
    hsT, wqkv_qk, wqkv_v = ins["hsT"], ins["wqkv_qk"], ins["wqkv_v"]
    bias_qk, bqkv_v_rep = ins["bias_qk"], ins["bqkv_v_rep"]
    ramp_rep, nslope_mat, bias_kq = ins["ramp_rep"], ins["nslope_mat"], ins["bias_kq"]
    wd = ins["wd"]
    out_part = outs["out_part"]

    TG = c.TG
    NG = c.NG

    # ---- persistent SBUF ----
    persist = ctx.enter_context(tc.tile_pool(name="persist", bufs=1))
    qkt_sb = persist.tile([128, c.QK_CT, c.TOKS], BF16, tag="qkt")
    v_sb = persist.tile([128, c.TOKS // 128, c.VC], BF16, tag="v")
    bias_qk_sb = persist.tile([128, c.QK_CT], F32, tag="bias_qk")
    bvrep_sb = persist.tile([128, c.VC], BF16, tag="bvrep")
    ramp_sb = persist.tile([128, 512], BF16, tag="ramp")
    nslope_sb = persist.tile([128, c.NPAIR * 128], BF16, tag="nslope")
    bias_kq_sb = persist.tile([128, c.NPAIR * c.NQT * c.CHQ], F32, tag="bias_kq")
    ones_sb = persist.tile([128, 32], BF16, tag="ones")

    nc.sync.dma_start(out=bias_qk_sb[:], in_=bias_qk[:])
    nc.sync.dma_start(out=bvrep_sb[:], in_=bqkv_v_rep[:])
    nc.sync.dma_start(out=ramp_sb[:], in_=ramp_rep[:])
    nc.sync.dma_start(out=nslope_sb[:], in_=nslope_mat[:])
    nc.sync.dma_start(out=bias_kq_sb[:], in_=bias_kq[:])
    nc.gpsimd.memset(ones_sb[:], 1.0)

    # ================= Phase 1: QKV projection =================
    sps_pool = ctx.enter_context(tc.tile_pool(name="u_sps", bufs=3, space="PSUM"))
    sum_pool = ctx.enter_context(tc.tile_pool(name="u_sum", bufs=1, space="PSUM"))
    ctx_pool = ctx.enter_context(tc.tile_pool(name="u_ctx", bufs=2, space="PSUM"))
    dps_pool = ctx.enter_context(tc.tile_pool(name="u_dps", bufs=2, space="PSUM"))

    _sid1, _ = nc.enter_named_scope("p1_qkv", False)
    with (
        tc.tile_pool(name="p1_hs", bufs=2) as hs_pool,
        tc.tile_pool(name="p1_w", bufs=3) as w_pool,
        tc.tile_pool(name="p1_wv", bufs=4) as wv_pool,
    ):
        hs_cur = hs_pool.tile([128, c.KT, TG], BF16, tag="hs")
        nc.gpsimd.dma_start(out=hs_cur[:], in_=hsT[0])
        for g in range(NG):
            g0 = g * TG
            hs_nxt = None
            if g + 1 < NG:
                hs_nxt = hs_pool.tile([128, c.KT, TG], BF16, tag="hs")
                nc.gpsimd.dma_start(out=hs_nxt[:], in_=hsT[g + 1])
            # wv prefetch pipeline (scalar queue), 2 tiles ahead
            wv_tiles = []
            for kt in range(2):
                wv = wv_pool.tile([128, c.VC], BF16, tag="wv")
                nc.scalar.dma_start(out=wv[:], in_=wqkv_v[:, ts(kt, c.VC)])
                wv_tiles.append(wv)
            # Q^T / K^T: out [col, tok]
            for ct in range(c.QK_CT):
                wst = w_pool.tile([128, c.HID], BF16, tag="wstripe")
                weng = nc.sync if ct % 2 == 0 else nc.scalar
                weng.dma_start(out=wst[:], in_=wqkv_qk[ct])
                qk_ps = sps_pool.tile([128, TG], F32, tag="s_ps")
                for kt in range(c.KT):
                    nc.tensor.matmul(
                        qk_ps[:],
                        wst[:, ts(kt, 128)],
                        hs_cur[:, kt, :],
                        start=(kt == 0), stop=(kt == c.KT - 1),
                    )
                nc.scalar.add(
                    qkt_sb[:, ct, g0:g0 + TG], qk_ps[:],
                    bias_qk_sb[:, ct:ct + 1],
                )
            # V: out [tok, vcol]; wqkv_v streamed per k-tile (kt-outer)
            v_pss = [
                (ctx_pool if tt < 2 else dps_pool).tile(
                    [128, c.VC], F32,
                    tag="ctx_ps" if tt < 2 else "d_ps",
                    name=f"v_ps{tt}",
                )
                for tt in range(TG // 128)
            ]
            for kt in range(c.KT):
                if kt + 2 < c.KT:
                    wv = wv_pool.tile([128, c.VC], BF16, tag="wv")
                    nc.scalar.dma_start(
                        out=wv[:], in_=wqkv_v[:, ts(kt + 2, c.VC)]
                    )
                    wv_tiles.append(wv)
                wv = wv_tiles.pop(0)
                for tt in range(TG // 128):
                    nc.tensor.matmul(
                        v_pss[tt][:],
                        hs_cur[:, kt, ts(tt, 128)],
                        wv[:],
                        start=(kt == 0), stop=(kt == c.KT - 1),
                    )
            for tt in range(TG // 128):
                nc.vector.tensor_tensor(
                    v_sb[:, g0 // 128 + tt, :], v_pss[tt][:], bvrep_sb[:],
                    ALU.add
                )
            hs_cur = hs_nxt
    nc.leave_named_scope("p1_qkv", _sid1, False)

    # ============ Phase 2+3: attention fused with dense ============
    # Transposed-score attention: scores^T [k, q]. Rank-1 matmul adds the
    # per-q alibi term in PSUM; Exp bias carries the per-k term; causal mask
    # is a post-exp fill-0 on GpSimd. Softmax sums via PE ones-reduce;
    # normalization folded into ctx evac via partition_broadcast.
    _sid2, _ = nc.enter_named_scope("p23_attn_dense", False)
    wd_persist = ctx.enter_context(tc.tile_pool(name="wd_persist", bufs=1))
    wd_sb = wd_persist.tile([128, c.MC, c.HID], BF16, tag="wd")
    for mc in range(c.MC):
        nc.sync.dma_start(out=wd_sb[:, mc, :], in_=wd[ts(mc, 128), :])
    with (
        tc.tile_pool(name="a_pt", bufs=1) as pt_pool,
        tc.tile_pool(name="a_sm", bufs=1) as sm_pool,
        tc.tile_pool(name="d_out", bufs=1) as dout_pool,
    ):
        for b in range(c.B):
            for cq in range(c.CHQ):
                WQ = 512
                q0 = cq * 512
                ktmax = 4 * (cq + 1)
                ctx_rolls = [
                    pt_pool.tile([128, 512], BF16, tag=f"ctx_roll{hl}",
                                 bufs=2, name=f"ctx_roll{hl}")
                    for hl in range(c.H_CORE)
                ]
                for hl in range(c.H_CORE):
                    p = b * c.H_CORE + hl
                    qT = qkt_sb[:, hl, b * c.S:(b + 1) * c.S]
                    kT = qkt_sb[:, c.H_CORE + hl, b * c.S:(b + 1) * c.S]
                    ctx_ps = ctx_pool.tile([128, 512], F32, tag="ctx_ps")
                    acc_e = sm_pool.tile([128, 512], BF16, tag="acc_e", bufs=2)
                    acc_o = sm_pool.tile([128, 512], BF16, tag="acc_o", bufs=2)

                    def stage_a(kt, p=p, cq=cq, q0=q0, qT=qT, kT=kT):
                        s_ps = sps_pool.tile([128, 512], F32, tag="s_ps")
                        nc.tensor.matmul(
                            s_ps[:, :WQ], nslope_sb[:, ts(p, 128)],
                            ramp_sb[:, :WQ],
                            start=True, stop=False,
                        )
                        nc.tensor.matmul(
                            s_ps[:, :WQ], kT[:, ts(kt, 128)],
                            qT[:, q0:q0 + WQ],
                            start=False, stop=True,
                        )
                        pt = pt_pool.tile([128, 512], BF16, tag="pt", bufs=8)
                        bidx = (p * c.NQT + kt) * c.CHQ + cq
                        nc.scalar.activation(
                            pt[:, :WQ], s_ps[:, :WQ], AF.Exp,
                            bias=bias_kq_sb[:, bidx:bidx + 1], scale=1.0,
                        )
                        dd = kt - 4 * cq
                        if dd >= 0:
                            # cols >= (dd+1)*128 always pass the causal test
                            mw = min(WQ, (dd + 1) * 128)
                            nc.gpsimd.affine_select(
                                pt[:, :mw], pt[:, :mw],
                                compare_op=ALU.is_ge, fill=0.0,
                                base=-dd * 128, pattern=[[1, mw]],
                                channel_multiplier=-1,
                            )
                        return pt

                    pts = {}

                    def stage_b(kt, pt, b=b, hl=hl, ctx_ps=ctx_ps,
                                acc_e=acc_e, acc_o=acc_o, pts=pts,
                                ktmax=ktmax):
                        st, sp = (kt == 0), (kt == ktmax - 1)
                        # softmax denominator: two interleaved bf16 running
                        # sums on DVE, partition-reduced once per chunk
                        acc = acc_e if kt % 2 == 0 else acc_o
                        if kt < 2:
                            pts[kt] = pt
                        elif kt < 4:
                            nc.vector.tensor_tensor(
                                acc[:, :WQ], pts.pop(kt - 2)[:, :WQ],
                                pt[:, :WQ], ALU.add,
                            )
                        else:
                            nc.vector.tensor_tensor(
                                acc[:, :WQ], acc[:, :WQ], pt[:, :WQ], ALU.add,
                            )
                        nc.tensor.matmul(
                            ctx_ps[:, :WQ],
                            v_sb[:, b * c.NQT + kt, ts(hl, 128)],
                            pt[:, :WQ],
                            start=st, stop=sp,
                        )

                    pend = []
                    for kt in range(ktmax):
                        pend.append((kt, stage_a(kt)))
                        if len(pend) > 3:
                            k0, pt0 = pend.pop(0)
                            stage_b(k0, pt0)
                    for k0, pt0 in pend:
                        stage_b(k0, pt0)

                    sum_ps = sum_pool.tile([32, 512], F32, tag="sum_ps")
                    nc.tensor.matmul(
                        sum_ps[:, :WQ], ones_sb[:], acc_e[:, :WQ],
                        start=True, stop=False,
                    )
                    nc.tensor.matmul(
                        sum_ps[:, :WQ], ones_sb[:], acc_o[:, :WQ],
                        start=False, stop=True,
                    )
                    rrow = sm_pool.tile([1, 512], F32, tag="rrow", bufs=2)
                    rrep = sm_pool.tile([128, 512], F32, tag="rrep", bufs=2)
                    nc.vector.reciprocal_approx_fast(
                        out=rrow[:, :WQ], in_=sum_ps[0:1, :WQ]
                    )
                    nc.gpsimd.partition_broadcast(rrep[:, :WQ], rrow[:, :WQ])
                    nc.vector.tensor_tensor(
                        ctx_rolls[hl][:, :WQ], ctx_ps[:, :WQ], rrep[:, :WQ],
                        ALU.mult,
                    )

                # dense for this q chunk's token tiles
                for sub in range(WQ // 128):
                    tt = b * c.NQT + cq * 4 + sub
                    o_sb = dout_pool.tile([128, c.HID], BF16, tag="o_sb",
                                          bufs=3)
                    for nb in range(c.HID // 512):
                        d_ps = dps_pool.tile([128, 512], F32, tag="d_ps")
                        for mc in range(c.MC):
                            nc.tensor.matmul(
                                d_ps[:],
                                ctx_rolls[mc][:, ts(sub, 128)],
                                wd_sb[:, mc, ts(nb, 512)],
                                start=(mc == 0), stop=(mc == c.MC - 1),
                            )
                        if nb % 2 == 0:
                            nc.scalar.copy(o_sb[:, ts(nb, 512)], d_ps[:])
                        else:
                            nc.vector.tensor_scalar(
                                o_sb[:, ts(nb, 512)], d_ps[:],
                                0.0, None, ALU.add,
                            )
                    nc.sync.dma_start(
                        out=out_part[ts(tt, 128), :], in_=o_sb[:]
                    )
    nc.leave_named_scope("p23_attn_dense", _sid2, False)


# ================= host side =================

def prep_shared(hidden_states, cfg):
    """hsT [NG, 128, KT, TG] bf16 — shared across cores, group-batched."""
    c = cfg
    hs = np.asarray(hidden_states, np.float32).reshape(c.TOKS, c.HID)
    hsT = np.ascontiguousarray(hs.T)                    # [HID, TOKS]
    t = hsT.reshape(c.KT, 128, c.NG, c.TG).transpose(2, 1, 0, 3)
    return np.ascontiguousarray(t).astype(BF)


def prep_core(alibi, Wqkv, bqkv, Wd, heads, cfg):
    """Per-core inputs for `heads` (list of H_CORE global head indices)."""
    c = cfg
    inv = 1.0 / math.sqrt(c.HD)
    Wq = np.asarray(Wqkv, np.float32).reshape(c.HID, -1, 3, c.HD)
    bq = np.asarray(bqkv, np.float32).reshape(-1, 3, c.HD)
    H = Wq.shape[1]

    # q cols pre-scaled by inv_norm; ct order: q heads then k heads
    w_q = Wq[:, heads, 0, :] * inv                      # [HID, H_CORE, HD]
    w_k = Wq[:, heads, 1, :]
    w_qk = np.concatenate([w_q, w_k], axis=1)           # [HID, QK_CT, 128]
    # -> [ct][hid_p][kt*128+col]: SBUF stripe rows are hid-within-chunk
    w_qk = w_qk.reshape(c.KT, 128, c.QK_CT, 128).transpose(2, 1, 0, 3)
    wqkv_qk = np.ascontiguousarray(w_qk.reshape(c.QK_CT, 128, c.HID)).astype(BF)

    w_v = Wq[:, heads, 2, :].reshape(c.HID, c.VC)       # [HID, VC]
    w_v = w_v.reshape(c.KT, 128, c.VC).transpose(1, 0, 2)  # [p, kt, vc]
    wqkv_v = np.ascontiguousarray(w_v.reshape(128, c.KT * c.VC)).astype(BF)

    b_q = bq[heads, 0, :] * inv                         # [H_CORE, 128]
    b_k = bq[heads, 1, :]
    b_qk = np.concatenate([b_q, b_k], axis=0)           # [QK_CT, 128]
    bias_qk = np.ascontiguousarray(b_qk.T).astype(np.float32)  # [128, QK_CT]

    b_v = bq[heads, 2, :].reshape(c.VC)
    bqkv_v_rep = np.ascontiguousarray(
        np.broadcast_to(b_v[None, :], (128, c.VC))
    ).astype(BF)

    al = np.asarray(alibi, np.float32).reshape(c.B, H, c.S)[:, heads]  # [B,HC,S]
    alibi_c = al.reshape(c.NPAIR, c.S).astype(np.float32)
    slope = alibi_c[:, 1] - alibi_c[:, 0]                  # [NPAIR]
    ramp_rep = np.ascontiguousarray(
        np.broadcast_to(np.arange(512, dtype=np.float32)[None, :], (128, 512))
    ).astype(BF)
    # uniform rank-1 stationary: every column of pair p's block is -slope/128
    nslope_mat = np.ascontiguousarray(
        np.broadcast_to(
            (-slope / 128.0)[None, :, None], (128, c.NPAIR, 128)
        ).reshape(128, c.NPAIR * 128)
    ).astype(BF)
    # exp bias per (pair, k-tile, q-chunk): alibi[k] - alibi[cq*512]
    bias_kq = np.zeros((128, c.NPAIR, c.NQT, c.CHQ), np.float32)
    for p in range(c.NPAIR):
        for kt in range(c.NQT):
            kpos = kt * 128 + np.arange(128)
            for cq in range(c.CHQ):
                bias_kq[:, p, kt, cq] = alibi_c[p, kpos] - alibi_c[p, cq * 512]
    bias_kq = np.ascontiguousarray(
        bias_kq.reshape(128, c.NPAIR * c.NQT * c.CHQ)
    )

    wd_c = np.asarray(Wd, np.float32).reshape(H, c.HD, c.HID)[heads]
    wd = np.ascontiguousarray(wd_c.reshape(c.MC * 128, c.HID)).astype(BF)

    return {
        "wqkv_qk": wqkv_qk,
        "wqkv_v": wqkv_v,
        "bias_qk": bias_qk,
        "bqkv_v_rep": bqkv_v_rep,
        "ramp_rep": ramp_rep,
        "nslope_mat": nslope_mat,
        "bias_kq": bias_kq,
        "wd": wd,
    }


def build_nc(cfg, debug=False):
    nc = bacc.Bacc("TRN2", target_bir_lowering=False, debug=debug)
    ins = {
        n: nc.dram_tensor(n, sh, dt, kind="ExternalInput").ap()
        for n, (sh, dt) in input_specs(cfg).items()
    }
    outs = {
        n: nc.dram_tensor(n, sh, dt, kind="ExternalOutput").ap()
        for n, (sh, dt) in output_specs(cfg).items()
    }
    with tile.TileContext(nc) as tc:
        with ExitStack() as es:
            build(es, tc, outs, ins, cfg)
    nc.compile()
    return nc


_NC_CACHE = {}


def _get_nc(cfg):
    if cfg not in _NC_CACHE:
        _NC_CACHE[cfg] = build_nc(cfg)
    return _NC_CACHE[cfg]


def _run(inputs, trace=False, **kwargs):
    cfg = FULL
    c = cfg
    hidden_states = np.asarray(inputs["hidden_states"], np.float32)
    residual = np.asarray(inputs["residual"], np.float32)
    alibi = np.asarray(inputs["alibi"], np.float32)
    Wqkv = np.asarray(inputs["Wqkv"], np.float32)
    bqkv = np.asarray(inputs["bqkv"], np.float32)
    Wd = np.asarray(inputs["Wd"], np.float32)
    bd = np.asarray(inputs["bd"], np.float32)

    nc = _get_nc(cfg)
    hsT = prep_shared(hidden_states, cfg)
    in_maps = []
    for core in range(N_CORES):
        heads = list(range(core * c.H_CORE, (core + 1) * c.H_CORE))
        m = {"hsT": hsT}
        m.update(prep_core(alibi, Wqkv, bqkv, Wd, heads, cfg))
        in_maps.append(m)

    res = run_bass_kernel_spmd(
        nc, in_maps, core_ids=list(range(N_CORES)), trace=trace, **kwargs
    )
    acc = np.zeros((c.TOKS, c.HID), np.float64)
    for r in res.results:
        acc += r["out_part"].astype(np.float64)
    out = acc.reshape(c.B, c.S, c.HID) + residual.astype(np.float64) + bd
    return out.astype(np.float32), res


def kernel(**inputs):
    out, _ = _run(inputs, trace=False)
    return out
